# revision 11
# baseline (speedup 1.0000x reference)
"""2-layer GAT (GATConv x2, PyG-style) on 8 Trainium2 NeuronCores — v3.

Strategy (edge-parallel, dst-sharded, slot-ordered, bf16 tables):
  - Nodes padded to 50176 and sharded contiguously: core c owns 6272 nodes
    (98 windows x 64 dst). Host permutes nodes into per-core slot order
    (windows sorted by edge count) so all cores run one SPMD program; the
    host un-permutes the output. Both layers share the same ordering, so
    one gather-index stream serves both.
  - Edges (incl. self loops) are bucketed by dst window; within a window
    they are split into "lo" blocks (src row < 25088) and "hi" blocks so
    each 128-edge block is src-half homogeneous. Bulk gathers use
    dma_gather (int16 indices, 256B rows): one lo + one hi gather per
    chunk of <=8 edge blocks (1024 descriptors; the SWDGE ring is grown
    to 64KB to keep two such gathers in flight).
  - Table rows are 128 bf16 (256B): cols 0:64 h (bf16), cols 64:64+2*H the
    per-src attention score sS as raw f32 bytes, rest unused. The per-dst
    score sD never travels through DRAM: it is broadcast to edges with a
    small PE matmul (transposed one-hot x sdw) per block.
  - Both one-hot matrices (oh: [edge, dst] scatter rhs; ohT: [dst, edge]
    broadcast lhsT) depend only on the host-known edge structure and are
    precomputed on the host (bf16 in DRAM, shared by both layers) and
    DMA-streamed per super-block — no on-device one-hot construction.
  - p = exp(leakyrelu(sS+sD)) (scores are bounded; no segment-max needed),
    messages m = h*p accumulate per dst via one-hot matmuls into PSUM.
    Layer-1 scatters transposed ([feat, dst]) so the normalized+ELU result
    lands directly as the lhsT of the layer-2 node matmul; layer-2
    scatters dst-major and stores straight to the output.
  - Softmax normalization + bias + activation are drained in a few big
    batched ops over a staging buffer, not per window.
  - AllGather outputs use the Shared DRAM address space (direct
    peer-writable scratchpad) for the fast collective path.
"""

import numpy as np
import ml_dtypes

BF = ml_dtypes.bfloat16

P = 128          # edges per block / SBUF partitions
W = 64           # dst nodes per window
NC = 8           # cores
WPC = 98         # windows per core
NPC = WPC * W    # nodes per core (6272)
NP = NC * NPC    # padded node count (50176)
HALF = NP // 2   # 25088: table half split for int16 gather indices
IN_DIM = 128
H1 = 8           # layer-1 heads
OUT_DIM = 64
NEG_SLOPE = 0.2
RLB = 128        # table row length in bf16 elems (256 bytes)
SUPER_BLK = 32   # max edge blocks per super
DCH = 8          # slots per drain chunk
GCH = 4          # edge blocks per dma_gather (512 descriptors = half the
                 # 16KB SWDGE ring, keeping two gathers in flight)
NCH = 4          # node-phase tiles per DMA chunk


def _mk_head_mat(a):
    """[H, C] attention vector -> [H*C, H] block-diagonal matrix."""
    H, C = a.shape
    A = np.zeros((H * C, H), np.float32)
    for h in range(H):
        A[h * C:(h + 1) * C, h] = a[h]
    return A


def _prep(x, edge_index, W1, a_src1, a_dst1, b1, W2, a_src2, a_dst2, b2,
          n_cores=NC, wpc=WPC):
    """Host-side preprocessing. Returns (cfg, in_maps, perm)."""
    npc = wpc * W
    n_pad = n_cores * npc
    n = x.shape[0]
    assert n <= n_pad

    x = np.asarray(x, np.float32)
    xp = np.zeros((n_pad, IN_DIM), np.float32)
    xp[:n] = x

    ei = np.asarray(edge_index)
    src = np.concatenate([ei[0], np.arange(n)]).astype(np.int64)
    dst = np.concatenate([ei[1], np.arange(n)]).astype(np.int64)

    win = (dst // W).astype(np.int64)
    half = (src >= HALF).astype(np.int64)
    # sort edges by (window, half); stable for determinism
    order = np.lexsort((half, win))
    src, dst, win, half = src[order], dst[order], win[order], half[order]
    nw = n_pad // W
    # counts per (window, half)
    key = win * 2 + half
    counts2 = np.bincount(key, minlength=nw * 2).reshape(nw, 2)
    starts2 = np.concatenate([[0], np.cumsum(counts2.reshape(-1))]).reshape(-1)

    counts_c = counts2.reshape(n_cores, wpc, 2)
    Klo_c = np.ceil(counts_c[:, :, 0] / P).astype(np.int64)
    Khi_c = np.ceil(counts_c[:, :, 1] / P).astype(np.int64)
    tot_c = counts_c.sum(axis=2)
    orders = [np.argsort(-tot_c[c], kind="stable") for c in range(n_cores)]
    Klo = np.max(np.stack([Klo_c[c][orders[c]] for c in range(n_cores)]), axis=0)
    Khi = np.max(np.stack([Khi_c[c][orders[c]] for c in range(n_cores)]), axis=0)
    # every window has self-loops so Klo+Khi >= 1 automatically
    assert (Klo + Khi).min() >= 1
    Mtot = int(Klo.sum() + Khi.sum())

    # perm[node] = global row in the slot-ordered table (same for both layers)
    perm = np.empty(n_pad, np.int64)
    for c in range(n_cores):
        inv = np.empty(wpc, np.int64)
        inv[orders[c]] = np.arange(wpc)
        wl = np.arange(wpc)
        base = (c * wpc + wl) * W
        for woff in range(W):
            perm[base + woff] = c * npc + inv * W + woff
    perm_inv = np.empty(n_pad, np.int64)
    perm_inv[perm] = np.arange(n_pad)

    # supers: greedy grouping of slots by block budget; within a super the
    # lo blocks of all slots come first, then the hi blocks.
    supers = []   # (sl0, nsl, bb0, nlo, nhi)
    block_slot = []   # global block id -> slot
    super_js = []     # per super: {slot: [local j list]}
    s = 0
    bb0 = 0
    while s < wpc:
        s0 = s
        nb = 0
        while s < wpc and nb + Klo[s] + Khi[s] <= SUPER_BLK:
            nb += int(Klo[s] + Khi[s])
            s += 1
        nsl = s - s0
        nlo = int(Klo[s0:s].sum())
        nhi = int(Khi[s0:s].sum())
        js = {}
        j = 0
        for sl in range(s0, s):
            js[sl] = list(range(j, j + int(Klo[sl])))
            j += int(Klo[sl])
        for sl in range(s0, s):
            js[sl] = js[sl] + list(range(j, j + int(Khi[sl])))
            j += int(Khi[sl])
        assert j == nlo + nhi == nb
        for sl in range(s0, s):
            for _ in range(int(Klo[sl])):
                block_slot.append(sl)
        for sl in range(s0, s):
            for _ in range(int(Khi[sl])):
                block_slot.append(sl)
        supers.append((s0, nsl, bb0, nlo, nhi))
        super_js.append(js)
        bb0 += nb
    assert bb0 == Mtot

    # per-core packed streams
    in_maps = []
    for c in range(n_cores):
        idx_vals = np.zeros((Mtot, P), np.int64)    # gather index per edge
        dstl_vals = np.full((Mtot, P), W, np.int64)  # local dst (W = pad)
        for si, (s0, nsl, b0, nlo, nhi) in enumerate(supers):
            js = super_js[si]
            for sl in range(s0, s0 + nsl):
                wloc = orders[c][sl]
                wglob = c * wpc + wloc
                for hf, Kh in ((0, Klo), (1, Khi)):
                    e0 = starts2[wglob * 2 + hf]
                    e1 = starts2[wglob * 2 + hf + 1]
                    ew = np.arange(e0, e1)
                    jlist = js[sl][:int(Klo[sl])] if hf == 0 else js[sl][int(Klo[sl]):]
                    for k, jl in enumerate(jlist):
                        blk = b0 + jl
                        seg = ew[k * P:(k + 1) * P]
                        m = seg.size
                        if m:
                            pv = perm[src[seg]]
                            idx_vals[blk, :m] = pv - (HALF if hf else 0)
                            dstl_vals[blk, :m] = dst[seg] - wglob * W
        # int16 wrapped index pack: edge (p, blk) -> [p%16 (+16g), blk*8 + p//16]
        iv = idx_vals.reshape(Mtot, 8, 16)           # [blk, q, r]
        idx16 = np.zeros((128, Mtot * 8), np.int16)
        base = np.transpose(iv, (2, 0, 1)).reshape(16, Mtot * 8)
        for g in range(8):
            idx16[g * 16:(g + 1) * 16, :] = base

        # host-built one-hot matrices (bf16), shared by both layers:
        #   oh  [P, Mtot*W]: oh[p, j*W + dstl[j,p]] = 1 (scatter rhs/lhsT)
        #   ohT [W, Mtot*P]: ohT[dstl[j,p], j*P + p] = 1 (sD broadcast lhsT)
        dl = dstl_vals.reshape(-1)
        jj = np.repeat(np.arange(Mtot), P)
        pp = np.tile(np.arange(P), Mtot)
        v = dl < W
        oh = np.zeros((P, Mtot * W), BF)
        oh[pp[v], jj[v] * W + dl[v]] = 1
        ohT = np.zeros((W, Mtot * P), BF)
        ohT[dl[v], jj[v] * P + pp[v]] = 1

        xs = xp[perm_inv[c * npc:(c + 1) * npc]]               # slot-ordered
        xT = np.ascontiguousarray(xs.T, np.float32)            # [128, npc]

        in_maps.append({
            "xT": xT,
            "idx": np.ascontiguousarray(idx16),
            "oh": np.ascontiguousarray(oh),
            "ohT": np.ascontiguousarray(ohT),
        })

    W1 = np.asarray(W1, np.float32)
    W2 = np.asarray(W2, np.float32)
    wc1 = np.concatenate(
        [W1, W1 @ _mk_head_mat(np.asarray(a_src1, np.float32)),
         W1 @ _mk_head_mat(np.asarray(a_dst1, np.float32))], axis=1)  # [128, 80]
    wc2 = np.concatenate(
        [W2, W2 @ np.asarray(a_src2, np.float32).T,
         W2 @ np.asarray(a_dst2, np.float32).T], axis=1)              # [64, 66]
    E8 = np.zeros((H1, W), np.float32)
    for h in range(H1):
        E8[h, h * 8:(h + 1) * 8] = 1.0
    b1c = np.asarray(b1, np.float32).reshape(W, 1)
    b2r = np.tile(np.asarray(b2, np.float32)[None, :], (W, 1))
    for m in in_maps:
        m["wc1"] = np.ascontiguousarray(wc1, np.float32)
        m["wc2"] = np.ascontiguousarray(wc2, np.float32)
        m["E8"] = np.ascontiguousarray(E8, np.float32)
        m["b1c"] = np.ascontiguousarray(b1c, np.float32)
        m["b2r"] = np.ascontiguousarray(b2r, np.float32)

    cfg = dict(n_cores=n_cores, wpc=wpc, npc=npc, n_pad=n_pad,
               supers=supers, super_js=super_js,
               block_slot=block_slot, Mtot=Mtot,
               Klo=[int(k) for k in Klo], Khi=[int(k) for k in Khi])
    return cfg, in_maps, perm


def _ap(base, off, dims):
    """Custom multi-level free-dim AP on top of a tile's [:, :] AP."""
    import concourse.bass as bass
    return bass.AP(tensor=base.tensor, offset=base.offset + off,
                   ap=[list(base.ap[0])] + [list(d) for d in dims])


def _build(nc, cfg, reps=1):
    import concourse.mybir as mybir
    import concourse.tile as tile
    from concourse.library_config import mlp

    f32 = mybir.dt.float32
    bf16 = mybir.dt.bfloat16
    i16 = mybir.dt.int16

    n_cores, wpc, npc, n_pad = cfg["n_cores"], cfg["wpc"], cfg["npc"], cfg["n_pad"]
    Mtot = cfg["Mtot"]
    nt = npc // P

    xT_d = nc.dram_tensor("xT", [P, npc], f32, kind="ExternalInput")
    idx_d = nc.dram_tensor("idx", [P, Mtot * 8], i16, kind="ExternalInput")
    oh_d = nc.dram_tensor("oh", [P, Mtot * W], bf16, kind="ExternalInput")
    ohT_d = nc.dram_tensor("ohT", [W, Mtot * P], bf16, kind="ExternalInput")
    wc1_d = nc.dram_tensor("wc1", [IN_DIM, 80], f32, kind="ExternalInput")
    wc2_d = nc.dram_tensor("wc2", [W, 66], f32, kind="ExternalInput")
    E8_d = nc.dram_tensor("E8", [H1, W], f32, kind="ExternalInput")
    b1c_d = nc.dram_tensor("b1c", [W, 1], f32, kind="ExternalInput")
    b2r_d = nc.dram_tensor("b2r", [W, OUT_DIM], f32, kind="ExternalInput")
    out_d = nc.dram_tensor("out", [npc, OUT_DIM], f32, kind="ExternalOutput")

    t1h_d = nc.dram_tensor("t1h", [npc, RLB], bf16, kind="Internal")
    tab1_d = nc.dram_tensor("tab1", [n_pad, RLB], bf16, kind="Internal")
    t2h_d = nc.dram_tensor("t2h", [npc, RLB], bf16, kind="Internal")
    tab2_d = nc.dram_tensor("tab2", [n_pad, RLB], bf16, kind="Internal")

    with tile.TileContext(nc) as tc:
        with tc.tile_pool(name="const", bufs=1) as cp, \
             tc.tile_pool(name="work", bufs=3) as wp, \
             tc.tile_pool(name="drain", bufs=2) as dp, \
             tc.tile_pool(name="gath", bufs=2) as gp, \
             tc.tile_pool(name="ohp", bufs=2) as op_, \
             tc.tile_pool(name="ohtp", bufs=2) as otp, \
             tc.tile_pool(name="nps", bufs=2, space="PSUM") as np_, \
             tc.tile_pool(name="sps", bufs=2, space="PSUM") as pp, \
             tc.tile_pool(name="sdp", bufs=2, space="PSUM") as sp:

            nc.gpsimd.load_library(mlp)

            def cload(name, shape, dt, src):
                t = cp.tile(shape, dt, tag=name)
                nc.sync.dma_start(out=t[:, :], in_=src[:, :])
                return t

            idx = cload("idx", [P, Mtot * 8], i16, idx_d)
            wc1 = cload("wc1", [IN_DIM, 80], f32, wc1_d)
            wc2 = cload("wc2", [W, 66], f32, wc2_d)
            E8 = cload("E8", [H1, W], f32, E8_d)
            b1c = cload("b1c", [W, 1], f32, b1c_d)
            b2r = cload("b2r", [W, OUT_DIM], f32, b2r_d)

            # sD tables in bf16 double-double: [hi | lo] per slot
            sdw1 = cp.tile([W, wpc * 2 * H1], bf16, tag="sdw1")
            sdw2 = cp.tile([W, wpc * 2], bf16, tag="sdw2")
            h2T = cp.tile([W, npc], f32, tag="h2T")

            env = dict(locals())
            for _rep in range(reps):
                _body(nc, cfg, env)


def _body(nc, cfg, env):
    import concourse.bass as bass
    import concourse.mybir as mybir

    f32 = mybir.dt.float32
    bf16 = mybir.dt.bfloat16
    Alu = mybir.AluOpType
    Act = mybir.ActivationFunctionType

    n_cores, wpc, npc, n_pad = cfg["n_cores"], cfg["wpc"], cfg["npc"], cfg["n_pad"]
    supers, super_js = cfg["supers"], cfg["super_js"]
    block_slot, Klo = cfg["block_slot"], cfg["Klo"]
    nt = npc // P
    groups = [list(range(n_cores))]

    import os as _os
    abl = _os.environ.get("K2_ABLATE", "")
    wp, gp, op_, otp = env["wp"], env["gp"], env["op_"], env["otp"]
    np_, pp, sp, cp = env["np_"], env["pp"], env["sp"], env["cp"]
    dp = env["dp"]
    idx, wc1, wc2 = env["idx"], env["wc1"], env["wc2"]
    E8, b1c, b2r = env["E8"], env["b1c"], env["b2r"]
    sdw1, sdw2, h2T = env["sdw1"], env["sdw2"], env["h2T"]
    xT_d, idx_d = env["xT_d"], env["idx_d"]
    oh_d, ohT_d = env["oh_d"], env["ohT_d"]
    t1h_d, tab1_d, t2h_d, tab2_d = (env["t1h_d"], env["tab1_d"], env["t2h_d"],
                                    env["tab2_d"])
    out_d = env["out_d"]

    # ---------------- layer-1 node phase ----------------
    for t in range(nt):
        xt = wp.tile([P, P], f32, tag="xt")
        nc.sync.dma_start(out=xt[:, :], in_=xT_d[:, t * P:(t + 1) * P])
        hp = np_.tile([P, 80], f32, tag="hp")
        nc.tensor.matmul(out=hp[:, :], lhsT=xt[:, :],
                         rhs=wc1[:, :], start=True, stop=True)
        st = wp.tile([P, RLB], bf16, tag="st")
        nc.vector.tensor_copy(out=st[:, :W], in_=hp[:, :W])
        stf = st[:, :].bitcast(f32)
        nc.vector.tensor_copy(
            out=bass.AP(tensor=stf.tensor, offset=stf.offset + 32,
                        ap=[list(stf.ap[0])] + [[1, H1]]),
            in_=hp[:, W:W + H1])
        nc.vector.memset(st[:, W + 2 * H1:], 0.0)
        for wi, rows in ((2 * t, slice(0, W)), (2 * t + 1, slice(W, P))):
            hi = sdw1[:, wi * 2 * H1:wi * 2 * H1 + H1]
            nc.vector.tensor_copy(out=hi, in_=hp[rows, 72:80])
            hiF = wp.tile([W, H1], f32, tag="hiF")
            nc.vector.tensor_copy(out=hiF[:, :], in_=hi)
            res = wp.tile([W, H1], f32, tag="res")
            nc.vector.tensor_tensor(out=res[:, :], in0=hp[rows, 72:80],
                                    in1=hiF[:, :], op=Alu.subtract)
            nc.vector.tensor_copy(
                out=sdw1[:, wi * 2 * H1 + H1:(wi + 1) * 2 * H1],
                in_=res[:, :])
        nc.sync.dma_start(out=t1h_d[t * P:(t + 1) * P, :], in_=st[:, :])

    if abl != "nocoll":
        nc.gpsimd.collective_compute(
            "AllGather", Alu.bypass, replica_groups=groups,
            ins=[t1h_d[:, :]], outs=[tab1_d[:, :]])

    # ---------------- layer-1 drain chunk (softmax div + bias + ELU) ----
    def flush1(chunk, ck, c):
        c0 = c * DCH * W
        den = dp.tile([H1, DCH * W], f32, tag="den")
        nc.vector.tensor_scalar_add(den[:, :ck], chunk[64:72, :ck], 1e-10)
        inv = dp.tile([H1, DCH * W], f32, tag="inv")
        nc.vector.reciprocal(inv[:, :ck], den[:, :ck])
        pb = np_.tile([W, DCH * W], f32, tag="pb")
        nc.tensor.matmul(out=pb[:, :ck], lhsT=E8[:, :], rhs=inv[:, :ck],
                         start=True, stop=True)
        ot = dp.tile([W, DCH * W], f32, tag="ot")
        nc.vector.tensor_tensor(out=ot[:, :ck], in0=chunk[:64, :ck],
                                in1=pb[:, :ck], op=Alu.mult)
        nc.vector.tensor_tensor(out=ot[:, :ck], in0=ot[:, :ck],
                                in1=b1c[:, :1].to_broadcast([W, ck]),
                                op=Alu.add)
        ex = dp.tile([W, DCH * W], f32, tag="ex")
        nc.scalar.activation(out=ex[:, :ck], in_=ot[:, :ck], func=Act.Exp)
        nc.vector.tensor_scalar(out=ex[:, :ck], in0=ex[:, :ck], scalar1=-1.0,
                                scalar2=0.0, op0=Alu.add, op1=Alu.min)
        rl = dp.tile([W, DCH * W], f32, tag="rl")
        nc.vector.tensor_scalar_max(rl[:, :ck], ot[:, :ck], 0.0)
        nc.vector.tensor_tensor(out=h2T[:, c0:c0 + ck], in0=ex[:, :ck],
                                in1=rl[:, :ck], op=Alu.add)

    # ---------------- layer-2 drain chunk (softmax div + bias + store) ----
    def flush2(chunk, ck, c):
        ns = ck // 65
        den2 = dp.tile([W, DCH], f32, tag="den2")
        nc.vector.tensor_scalar_add(
            den2[:, :ns], _ap(chunk[:, :], 64, [[65, ns]]), 1e-10)
        inv2 = dp.tile([W, DCH], f32, tag="inv2")
        nc.vector.reciprocal(inv2[:, :ns], den2[:, :ns])
        ob = dp.tile([W, DCH * OUT_DIM], f32, tag="ob")
        nc.vector.tensor_tensor(
            out=_ap(ob[:, :], 0, [[OUT_DIM, ns], [1, OUT_DIM]]),
            in0=_ap(chunk[:, :], 0, [[65, ns], [1, OUT_DIM]]),
            in1=_ap(inv2[:, :], 0, [[1, ns], [0, OUT_DIM]]),
            op=Alu.mult)
        nc.vector.tensor_tensor(
            out=_ap(ob[:, :], 0, [[OUT_DIM, ns], [1, OUT_DIM]]),
            in0=_ap(ob[:, :], 0, [[OUT_DIM, ns], [1, OUT_DIM]]),
            in1=_ap(b2r[:, :], 0, [[0, ns], [1, OUT_DIM]]),
            op=Alu.add)
        nc.sync.dma_start(
            out=bass.AP(tensor=out_d[:, :].tensor,
                        offset=c * DCH * W * OUT_DIM,
                        ap=[[OUT_DIM, W], [W * OUT_DIM, ns], [1, OUT_DIM]]),
            in_=ob[:, :ns * OUT_DIM])

    # ---------------- edge phase ----------------
    def edge_phase(layer, tab, H, GWm, sdw, scol, flush):
        chunk = None
        for si, (sl0, nsl, bb0, nlo, nhi) in enumerate(supers):
            nblk = nlo + nhi
            js = super_js[si]
            G = gp.tile([P, nblk * RLB], bf16, tag="G")

            def out3(apx, k):
                return bass.AP(tensor=apx.tensor, offset=apx.offset,
                               ap=[list(apx.ap[0])] + [[RLB, k], [1, RLB]])

            def gathers(col0, nb, tab_slice):
                for off in range(0, nb, GCH):
                    k = min(GCH, nb - off)
                    a = col0 + off
                    nc.gpsimd.dma_gather(
                        out_ap=out3(G[:, a * RLB:(a + k) * RLB], k),
                        in_ap=tab_slice,
                        idxs_ap=idx[:, (bb0 + a) * 8:(bb0 + a + k) * 8],
                        num_idxs=k * P, num_idxs_reg=k * P, elem_size=RLB)

            if abl != "nogather":
                if nlo:
                    gathers(0, nlo, tab[0:HALF, :])
                if nhi:
                    gathers(nlo, nhi, tab[HALF:, :])

            # host-precomputed one-hots, streamed from DRAM
            ohT = otp.tile([W, nblk * P], bf16, tag="ohT")
            nc.sync.dma_start(out=ohT[:, :],
                              in_=ohT_d[:, bb0 * P:(bb0 + nblk) * P])
            oh = op_.tile([P, nblk * W], bf16, tag="oh")
            nc.sync.dma_start(out=oh[:, :],
                              in_=oh_d[:, bb0 * W:(bb0 + nblk) * W])

            # per-edge sD via transposed-one-hot matmuls ([hi | lo] rhs)
            psD = sp.tile([P, nblk * 2 * H], f32, tag="psD")
            if abl != "nopsd":
                for j in range(nblk):
                    s = block_slot[bb0 + j]
                    nc.tensor.matmul(
                        out=psD[:, j * 2 * H:(j + 1) * 2 * H],
                        lhsT=ohT[:, j * P:(j + 1) * P],
                        rhs=sdw[:, s * 2 * H:(s + 1) * 2 * H],
                        start=True, stop=True)

            # e = sS + sD_hi + sD_lo ; lrelu ; p = exp -> G score cols (bf16)
            Gf = G[:, :].bitcast(f32)
            sS = bass.AP(tensor=Gf.tensor, offset=Gf.offset + 32,
                         ap=[list(Gf.ap[0])] + [[RLB // 2, nblk], [1, H]])
            eS = wp.tile([P, nblk * H], f32, tag="eS")
            if abl == "nopsd":
                nc.vector.tensor_copy(out=eS[:, :], in_=sS)
            else:
                nc.vector.tensor_tensor(
                    out=eS[:, :], in0=sS,
                    in1=_ap(psD[:, :], 0, [[2 * H, nblk], [1, H]]), op=Alu.add)
                nc.vector.tensor_tensor(
                    out=eS[:, :], in0=eS[:, :],
                    in1=_ap(psD[:, :], H, [[2 * H, nblk], [1, H]]), op=Alu.add)
            nc.vector.scalar_tensor_tensor(
                out=eS[:, :], in0=eS[:, :], scalar=NEG_SLOPE,
                in1=eS[:, :], op0=Alu.mult, op1=Alu.max)
            p_dst = _ap(G[:, :], W, [[RLB, nblk], [1, H]])
            nc.scalar.activation(out=p_dst, in_=eS[:, :], func=Act.Exp)

            # msg = h * p (per-head broadcast)
            if H == 1:
                m_ap = _ap(G[:, :], 0, [[RLB, nblk], [1, W]])
                p_ap = _ap(G[:, :], W, [[RLB, nblk], [0, W]])
            else:
                m_ap = _ap(G[:, :], 0, [[RLB, nblk], [H, H], [1, 64 // H]])
                p_ap = _ap(G[:, :], W, [[RLB, nblk], [1, H], [0, 64 // H]])
            nc.vector.tensor_tensor(out=m_ap, in0=m_ap, in1=p_ap, op=Alu.mult)

            # scatter per slot; stage into drain chunks of DCH slots
            for s in range(sl0, sl0 + nsl):
                jl = js[s]
                if layer == 1:
                    ps = pp.tile([64 + H, W], f32, tag="ps")
                    for k, j in enumerate(jl):
                        nc.tensor.matmul(
                            out=ps[:, :],
                            lhsT=_ap(G[:, :], j * RLB, [[1, GWm]]),
                            rhs=oh[:, j * W:(j + 1) * W],
                            start=(k == 0), stop=(k == len(jl) - 1))
                    rows = 64 + H
                else:
                    ps = pp.tile([W, GWm], f32, tag="ps")
                    for k, j in enumerate(jl):
                        nc.tensor.matmul(
                            out=ps[:, :],
                            lhsT=oh[:, j * W:(j + 1) * W],
                            rhs=_ap(G[:, :], j * RLB, [[1, GWm]]),
                            start=(k == 0), stop=(k == len(jl) - 1))
                    rows = W
                if chunk is None:
                    chunk = dp.tile([rows, DCH * scol], f32,
                                    tag=f"stage{layer}")
                sloc = s % DCH
                nc.vector.tensor_copy(
                    out=chunk[:, sloc * scol:(sloc + 1) * scol], in_=ps[:, :])
                if sloc == DCH - 1 or s == wpc - 1:
                    flush(chunk, (sloc + 1) * scol, s // DCH)
                    chunk = None

    edge_phase(1, tab1_d, H1, 64 + H1, sdw1, W, flush1)

    # ---------------- layer-2 node phase ----------------
    for t in range(nt):
        hp2 = np_.tile([P, 80], f32, tag="hp")
        nc.tensor.matmul(out=hp2[:, :66], lhsT=h2T[:, t * P:(t + 1) * P],
                         rhs=wc2[:, :], start=True, stop=True)
        st2 = wp.tile([P, RLB], bf16, tag="st2")
        nc.vector.tensor_copy(out=st2[:, :W], in_=hp2[:, :W])
        stf2 = st2[:, :].bitcast(f32)
        nc.vector.tensor_copy(
            out=bass.AP(tensor=stf2.tensor, offset=stf2.offset + 32,
                        ap=[list(stf2.ap[0])] + [[1, 1]]),
            in_=hp2[:, W:W + 1])
        nc.vector.memset(st2[:, W + 2:], 0.0)
        for wi, rows in ((2 * t, slice(0, W)), (2 * t + 1, slice(W, P))):
            hi = sdw2[:, wi * 2:wi * 2 + 1]
            nc.vector.tensor_copy(out=hi, in_=hp2[rows, 65:66])
            hiF = wp.tile([W, 1], f32, tag="hiF2")
            nc.vector.tensor_copy(out=hiF[:, :], in_=hi)
            res = wp.tile([W, 1], f32, tag="res2")
            nc.vector.tensor_tensor(out=res[:, :], in0=hp2[rows, 65:66],
                                    in1=hiF[:, :], op=Alu.subtract)
            nc.vector.tensor_copy(out=sdw2[:, wi * 2 + 1:wi * 2 + 2],
                                  in_=res[:, :])
        nc.sync.dma_start(out=t2h_d[t * P:(t + 1) * P, :], in_=st2[:, :])

    if abl != "nocoll":
        nc.gpsimd.collective_compute(
            "AllGather", Alu.bypass, replica_groups=groups,
            ins=[t2h_d[:, :]], outs=[tab2_d[:, :]])

    edge_phase(2, tab2_d, 1, 65, sdw2, 65, flush2)


def kernel(**inputs):
    import concourse.bacc as bacc
    from concourse.bass_utils import run_bass_kernel_spmd

    n = inputs["x"].shape[0]
    cfg, in_maps, perm = _prep(**inputs)

    nc = bacc.Bacc("TRN2", target_bir_lowering=False, debug=False,
                   num_devices=cfg["n_cores"])
    _build(nc, cfg)
    nc.compile()

    res = run_bass_kernel_spmd(nc, in_maps,
                               core_ids=list(range(cfg["n_cores"])))
    full = np.concatenate([r["out"] for r in res.results], axis=0)
    out = full[perm[:n]]
    return np.ascontiguousarray(out, np.float32)


# revision 17
# speedup vs baseline: 1.0156x; 1.0156x over previous
"""2-layer GAT (GATConv x2, PyG-style) on 8 Trainium2 NeuronCores — v3.

Strategy (edge-parallel, dst-sharded, slot-ordered, bf16 tables):
  - Nodes padded to 50176 and sharded contiguously: core c owns 6272 nodes
    (98 windows x 64 dst). Host permutes nodes into per-core slot order
    (windows sorted by edge count) so all cores run one SPMD program; the
    host un-permutes the output. Both layers share the same ordering, so
    one gather-index stream serves both.
  - Edges (incl. self loops) are bucketed by dst window; within a window
    they are split into "lo" blocks (src row < 25088) and "hi" blocks so
    each 128-edge block is src-half homogeneous. Bulk gathers use
    dma_gather (int16 indices, 256B rows): one lo + one hi gather per
    chunk of <=8 edge blocks (1024 descriptors; the SWDGE ring is grown
    to 64KB to keep two such gathers in flight).
  - Table rows are 128 bf16 (256B): cols 0:64 h (bf16), cols 64:64+2*H the
    per-src attention score sS as raw f32 bytes, rest unused. The per-dst
    score sD never travels through DRAM: it is broadcast to edges with a
    small PE matmul (transposed one-hot x sdw) per block.
  - Both one-hot matrices (oh: [edge, dst] scatter rhs; ohT: [dst, edge]
    broadcast lhsT) depend only on the host-known edge structure and are
    precomputed on the host (bf16 in DRAM, shared by both layers) and
    DMA-streamed per super-block — no on-device one-hot construction.
  - p = exp(leakyrelu(sS+sD)) (scores are bounded; no segment-max needed),
    messages m = h*p accumulate per dst via one-hot matmuls into PSUM.
    Layer-1 scatters transposed ([feat, dst]) so the normalized+ELU result
    lands directly as the lhsT of the layer-2 node matmul; layer-2
    scatters dst-major and stores straight to the output.
  - Softmax normalization + bias + activation are drained in a few big
    batched ops over a staging buffer, not per window.
  - AllGather outputs use the Shared DRAM address space (direct
    peer-writable scratchpad) for the fast collective path.
"""

import numpy as np
import ml_dtypes

BF = ml_dtypes.bfloat16

P = 128          # edges per block / SBUF partitions
W = 64           # dst nodes per window
NC = 8           # cores
WPC = 98         # windows per core
NPC = WPC * W    # nodes per core (6272)
NP = NC * NPC    # padded node count (50176)
HALF = NP // 2   # 25088: table half split for int16 gather indices
IN_DIM = 128
H1 = 8           # layer-1 heads
OUT_DIM = 64
NEG_SLOPE = 0.2
RLB = 128        # table row length in bf16 elems (256 bytes)
SUPER_BLK = 32   # max edge blocks per super
DCH = 8          # slots per drain chunk
GCH = 4          # edge blocks per dma_gather (512 descriptors = half the
                 # 16KB SWDGE ring, keeping two gathers in flight)
NCH = 7          # node-phase tiles per DMA chunk (49 = 7x7 tiles)


def _mk_head_mat(a):
    """[H, C] attention vector -> [H*C, H] block-diagonal matrix."""
    H, C = a.shape
    A = np.zeros((H * C, H), np.float32)
    for h in range(H):
        A[h * C:(h + 1) * C, h] = a[h]
    return A


def _prep(x, edge_index, W1, a_src1, a_dst1, b1, W2, a_src2, a_dst2, b2,
          n_cores=NC, wpc=WPC):
    """Host-side preprocessing. Returns (cfg, in_maps, perm)."""
    npc = wpc * W
    n_pad = n_cores * npc
    n = x.shape[0]
    assert n <= n_pad

    x = np.asarray(x, np.float32)
    xp = np.zeros((n_pad, IN_DIM), np.float32)
    xp[:n] = x

    ei = np.asarray(edge_index)
    src = np.concatenate([ei[0], np.arange(n)]).astype(np.int64)
    dst = np.concatenate([ei[1], np.arange(n)]).astype(np.int64)

    win = (dst // W).astype(np.int64)
    half = (src >= HALF).astype(np.int64)
    # sort edges by (window, half); stable for determinism
    order = np.lexsort((half, win))
    src, dst, win, half = src[order], dst[order], win[order], half[order]
    nw = n_pad // W
    # counts per (window, half)
    key = win * 2 + half
    counts2 = np.bincount(key, minlength=nw * 2).reshape(nw, 2)
    starts2 = np.concatenate([[0], np.cumsum(counts2.reshape(-1))]).reshape(-1)

    counts_c = counts2.reshape(n_cores, wpc, 2)
    Klo_c = np.ceil(counts_c[:, :, 0] / P).astype(np.int64)
    Khi_c = np.ceil(counts_c[:, :, 1] / P).astype(np.int64)
    tot_c = counts_c.sum(axis=2)
    orders = [np.argsort(-tot_c[c], kind="stable") for c in range(n_cores)]
    Klo = np.max(np.stack([Klo_c[c][orders[c]] for c in range(n_cores)]), axis=0)
    Khi = np.max(np.stack([Khi_c[c][orders[c]] for c in range(n_cores)]), axis=0)
    # every window has self-loops so Klo+Khi >= 1 automatically
    assert (Klo + Khi).min() >= 1
    Mtot = int(Klo.sum() + Khi.sum())

    # perm[node] = global row in the slot-ordered table (same for both layers)
    perm = np.empty(n_pad, np.int64)
    for c in range(n_cores):
        inv = np.empty(wpc, np.int64)
        inv[orders[c]] = np.arange(wpc)
        wl = np.arange(wpc)
        base = (c * wpc + wl) * W
        for woff in range(W):
            perm[base + woff] = c * npc + inv * W + woff
    perm_inv = np.empty(n_pad, np.int64)
    perm_inv[perm] = np.arange(n_pad)

    # supers: greedy grouping of slots by block budget; within a super the
    # lo blocks of all slots come first, then the hi blocks.
    supers = []   # (sl0, nsl, bb0, nlo, nhi)
    block_slot = []   # global block id -> slot
    super_js = []     # per super: {slot: [local j list]}
    s = 0
    bb0 = 0
    while s < wpc:
        s0 = s
        nb = 0
        while s < wpc and nb + Klo[s] + Khi[s] <= SUPER_BLK:
            nb += int(Klo[s] + Khi[s])
            s += 1
        nsl = s - s0
        nlo = int(Klo[s0:s].sum())
        nhi = int(Khi[s0:s].sum())
        js = {}
        j = 0
        for sl in range(s0, s):
            js[sl] = list(range(j, j + int(Klo[sl])))
            j += int(Klo[sl])
        for sl in range(s0, s):
            js[sl] = js[sl] + list(range(j, j + int(Khi[sl])))
            j += int(Khi[sl])
        assert j == nlo + nhi == nb
        for sl in range(s0, s):
            for _ in range(int(Klo[sl])):
                block_slot.append(sl)
        for sl in range(s0, s):
            for _ in range(int(Khi[sl])):
                block_slot.append(sl)
        supers.append((s0, nsl, bb0, nlo, nhi))
        super_js.append(js)
        bb0 += nb
    assert bb0 == Mtot

    # per-core packed streams
    in_maps = []
    for c in range(n_cores):
        idx_vals = np.zeros((Mtot, P), np.int64)    # gather index per edge
        dstl_vals = np.full((Mtot, P), W, np.int64)  # local dst (W = pad)
        for si, (s0, nsl, b0, nlo, nhi) in enumerate(supers):
            js = super_js[si]
            for sl in range(s0, s0 + nsl):
                wloc = orders[c][sl]
                wglob = c * wpc + wloc
                for hf, Kh in ((0, Klo), (1, Khi)):
                    e0 = starts2[wglob * 2 + hf]
                    e1 = starts2[wglob * 2 + hf + 1]
                    ew = np.arange(e0, e1)
                    jlist = js[sl][:int(Klo[sl])] if hf == 0 else js[sl][int(Klo[sl]):]
                    for k, jl in enumerate(jlist):
                        blk = b0 + jl
                        seg = ew[k * P:(k + 1) * P]
                        m = seg.size
                        if m:
                            pv = perm[src[seg]]
                            idx_vals[blk, :m] = pv - (HALF if hf else 0)
                            dstl_vals[blk, :m] = dst[seg] - wglob * W
        # int16 wrapped index pack: edge (p, blk) -> [p%16 (+16g), blk*8 + p//16]
        iv = idx_vals.reshape(Mtot, 8, 16)           # [blk, q, r]
        idx16 = np.zeros((128, Mtot * 8), np.int16)
        base = np.transpose(iv, (2, 0, 1)).reshape(16, Mtot * 8)
        for g in range(8):
            idx16[g * 16:(g + 1) * 16, :] = base

        # host-built one-hot matrices (bf16), shared by both layers:
        #   oh  [P, Mtot*W]: oh[p, j*W + dstl[j,p]] = 1 (scatter rhs/lhsT)
        #   ohT [W, Mtot*P]: ohT[dstl[j,p], j*P + p] = 1 (sD broadcast lhsT)
        dl = dstl_vals.reshape(-1)
        jj = np.repeat(np.arange(Mtot), P)
        pp = np.tile(np.arange(P), Mtot)
        v = dl < W
        oh = np.zeros((P, Mtot * W), BF)
        oh[pp[v], jj[v] * W + dl[v]] = 1
        ohT = np.zeros((W, Mtot * P), BF)
        ohT[dl[v], jj[v] * P + pp[v]] = 1

        xs = xp[perm_inv[c * npc:(c + 1) * npc]]               # slot-ordered
        xT = np.ascontiguousarray(xs.T, np.float32)            # [128, npc]

        in_maps.append({
            "xT": xT,
            "idx": np.ascontiguousarray(idx16),
            "oh": np.ascontiguousarray(oh),
            "ohT": np.ascontiguousarray(ohT),
        })

    # full slot-ordered feature table, identical on every core (the layer-1
    # node phase is replicated so no AllGather is needed for tab1)
    xTf = np.ascontiguousarray(xp[perm_inv].T, np.float32)     # [128, n_pad]
    for m in in_maps:
        m["xTf"] = xTf

    W1 = np.asarray(W1, np.float32)
    W2 = np.asarray(W2, np.float32)
    wc1 = np.concatenate(
        [W1, W1 @ _mk_head_mat(np.asarray(a_src1, np.float32)),
         W1 @ _mk_head_mat(np.asarray(a_dst1, np.float32))], axis=1)  # [128, 80]
    wc2 = np.concatenate(
        [W2, W2 @ np.asarray(a_src2, np.float32).T,
         W2 @ np.asarray(a_dst2, np.float32).T], axis=1)              # [64, 66]
    E8 = np.zeros((H1, W), np.float32)
    for h in range(H1):
        E8[h, h * 8:(h + 1) * 8] = 1.0
    b1c = np.asarray(b1, np.float32).reshape(W, 1)
    b2r = np.tile(np.asarray(b2, np.float32)[None, :], (W, 1))
    # hi/lo split of W2 for the bf16 replicated layer-2 transform
    w2hi = W2.astype(BF)
    w2lo = (W2 - w2hi.astype(np.float32)).astype(BF)
    wc2hl = np.concatenate([w2hi, w2lo], axis=1)               # [64, 128]
    for m in in_maps:
        m["wc1"] = np.ascontiguousarray(wc1, np.float32)
        m["wc2"] = np.ascontiguousarray(wc2, np.float32)
        m["wc2hl"] = np.ascontiguousarray(wc2hl, BF)
        m["E8"] = np.ascontiguousarray(E8, np.float32)
        m["b1c"] = np.ascontiguousarray(b1c, np.float32)
        m["b2r"] = np.ascontiguousarray(b2r, np.float32)

    cfg = dict(n_cores=n_cores, wpc=wpc, npc=npc, n_pad=n_pad,
               supers=supers, super_js=super_js,
               block_slot=block_slot, Mtot=Mtot,
               Klo=[int(k) for k in Klo], Khi=[int(k) for k in Khi])
    return cfg, in_maps, perm


def _ap(base, off, dims):
    """Custom multi-level free-dim AP on top of a tile's [:, :] AP."""
    import concourse.bass as bass
    return bass.AP(tensor=base.tensor, offset=base.offset + off,
                   ap=[list(base.ap[0])] + [list(d) for d in dims])


def _build(nc, cfg, reps=1):
    import concourse.mybir as mybir
    import concourse.tile as tile
    from concourse.library_config import mlp

    f32 = mybir.dt.float32
    bf16 = mybir.dt.bfloat16
    i16 = mybir.dt.int16

    n_cores, wpc, npc, n_pad = cfg["n_cores"], cfg["wpc"], cfg["npc"], cfg["n_pad"]
    Mtot = cfg["Mtot"]
    nt = npc // P

    xT_d = nc.dram_tensor("xT", [P, npc], f32, kind="ExternalInput")
    xTf_d = nc.dram_tensor("xTf", [P, n_pad], f32, kind="ExternalInput")
    idx_d = nc.dram_tensor("idx", [P, Mtot * 8], i16, kind="ExternalInput")
    oh_d = nc.dram_tensor("oh", [P, Mtot * W], bf16, kind="ExternalInput")
    ohT_d = nc.dram_tensor("ohT", [W, Mtot * P], bf16, kind="ExternalInput")
    wc1_d = nc.dram_tensor("wc1", [IN_DIM, 80], f32, kind="ExternalInput")
    wc2_d = nc.dram_tensor("wc2", [W, 66], f32, kind="ExternalInput")
    wc2hl_d = nc.dram_tensor("wc2hl", [W, 2 * W], bf16, kind="ExternalInput")
    E8_d = nc.dram_tensor("E8", [H1, W], f32, kind="ExternalInput")
    b1c_d = nc.dram_tensor("b1c", [W, 1], f32, kind="ExternalInput")
    b2r_d = nc.dram_tensor("b2r", [W, OUT_DIM], f32, kind="ExternalInput")
    out_d = nc.dram_tensor("out", [npc, OUT_DIM], f32, kind="ExternalOutput")

    tab1_d = nc.dram_tensor("tab1", [n_pad, RLB], bf16, kind="Internal")
    t2h_d = nc.dram_tensor("t2h", [W, npc], bf16, kind="Internal")
    h2g_d = nc.dram_tensor("h2g", [n_cores * W, npc], bf16, kind="Internal",
                           addr_space="Shared")
    sc2_d = nc.dram_tensor("sc2", [P, npc // P], f32, kind="Internal")
    sc2g_d = nc.dram_tensor("sc2g", [n_cores * P, npc // P], f32,
                            kind="Internal", addr_space="Shared")
    tab2_d = nc.dram_tensor("tab2", [n_pad, RLB], bf16, kind="Internal")

    with tile.TileContext(nc) as tc:
        with tc.tile_pool(name="const", bufs=1) as cp, \
             tc.tile_pool(name="work", bufs=3) as wp, \
             tc.tile_pool(name="drain", bufs=2) as dp, \
             tc.tile_pool(name="gath", bufs=2) as gp, \
             tc.tile_pool(name="ohp", bufs=2) as op_, \
             tc.tile_pool(name="ohtp", bufs=2) as otp, \
             tc.tile_pool(name="nps", bufs=2, space="PSUM") as np_, \
             tc.tile_pool(name="sps", bufs=2, space="PSUM") as pp, \
             tc.tile_pool(name="sdp", bufs=2, space="PSUM") as sp:

            nc.gpsimd.load_library(mlp)

            def cload(name, shape, dt, src):
                t = cp.tile(shape, dt, tag=name)
                nc.sync.dma_start(out=t[:, :], in_=src[:, :])
                return t

            idx = cload("idx", [P, Mtot * 8], i16, idx_d)
            wc1 = cload("wc1", [IN_DIM, 80], f32, wc1_d)
            wc2 = cload("wc2", [W, 66], f32, wc2_d)
            wc2hl = cload("wc2hl", [W, 2 * W], bf16, wc2hl_d)
            E8 = cload("E8", [H1, W], f32, E8_d)
            b1c = cload("b1c", [W, 1], f32, b1c_d)
            b2r = cload("b2r", [W, OUT_DIM], f32, b2r_d)

            # sD tables in bf16 double-double: [hi | lo] per slot
            sdw1 = cp.tile([W, wpc * 2 * H1], bf16, tag="sdw1")
            sdw2 = cp.tile([W, wpc * 2], bf16, tag="sdw2")
            h2T = cp.tile([W, npc], f32, tag="h2T")
            h2b = cp.tile([W, npc], bf16, tag="h2b")
            sdS1 = cp.tile([P, (npc // P) * H1], f32, tag="sdS1")
            sdF1 = cp.tile([W, wpc * H1], f32, tag="sdF1")
            hiF1 = cp.tile([W, wpc * H1], f32, tag="hiF1")
            scS = cp.tile([P, npc // P], f32, tag="scS")
            sdS2 = cp.tile([P, npc // P], f32, tag="sdS2")
            sdF2 = cp.tile([W, wpc], f32, tag="sdF2")
            hiF2 = cp.tile([W, wpc], f32, tag="hiF2")

            env = dict(locals())
            for _rep in range(reps):
                _body(nc, cfg, env)


def _body(nc, cfg, env):
    import concourse.bass as bass
    import concourse.mybir as mybir

    f32 = mybir.dt.float32
    bf16 = mybir.dt.bfloat16
    Alu = mybir.AluOpType
    Act = mybir.ActivationFunctionType

    n_cores, wpc, npc, n_pad = cfg["n_cores"], cfg["wpc"], cfg["npc"], cfg["n_pad"]
    supers, super_js = cfg["supers"], cfg["super_js"]
    block_slot, Klo = cfg["block_slot"], cfg["Klo"]
    nt = npc // P
    groups = [list(range(n_cores))]

    import os as _os
    abl = _os.environ.get("K2_ABLATE", "")
    wp, gp, op_, otp = env["wp"], env["gp"], env["op_"], env["otp"]
    np_, pp, sp, cp = env["np_"], env["pp"], env["sp"], env["cp"]
    dp = env["dp"]
    idx, wc1, wc2 = env["idx"], env["wc1"], env["wc2"]
    E8, b1c, b2r = env["E8"], env["b1c"], env["b2r"]
    sdw1, sdw2, h2T = env["sdw1"], env["sdw2"], env["h2T"]
    xT_d, idx_d = env["xT_d"], env["idx_d"]
    oh_d, ohT_d = env["oh_d"], env["ohT_d"]
    t1h_d, tab1_d, t2h_d, tab2_d = (env["t1h_d"], env["tab1_d"], env["t2h_d"],
                                    env["tab2_d"])
    out_d = env["out_d"]

    # ---------------- layer-1 node phase ----------------
    for tc_ in range(nt // NCH):
        xt = wp.tile([P, NCH * P], f32, tag="xt")
        nc.sync.dma_start(out=xt[:, :],
                          in_=xT_d[:, tc_ * NCH * P:(tc_ + 1) * NCH * P])
        st = wp.tile([P, NCH * RLB], bf16, tag="st")
        for q in range(NCH):
            t = tc_ * NCH + q
            hp = np_.tile([P, 80], f32, tag="hp")
            nc.tensor.matmul(out=hp[:, :], lhsT=xt[:, q * P:(q + 1) * P],
                             rhs=wc1[:, :], start=True, stop=True)
            nc.vector.tensor_copy(out=st[:, q * RLB:q * RLB + W],
                                  in_=hp[:, :W])
            stf = st[:, :].bitcast(f32)
            nc.vector.tensor_copy(
                out=bass.AP(tensor=stf.tensor,
                            offset=stf.offset + q * (RLB // 2) + 32,
                            ap=[list(stf.ap[0])] + [[1, H1]]),
                in_=hp[:, W:W + H1])
            nc.vector.memset(st[:, q * RLB + W + 2 * H1:(q + 1) * RLB], 0.0)
            for wi, rows in ((2 * t, slice(0, W)), (2 * t + 1, slice(W, P))):
                hi = sdw1[:, wi * 2 * H1:wi * 2 * H1 + H1]
                nc.vector.tensor_copy(out=hi, in_=hp[rows, 72:80])
                hiF = wp.tile([W, H1], f32, tag="hiF")
                nc.vector.tensor_copy(out=hiF[:, :], in_=hi)
                res = wp.tile([W, H1], f32, tag="res")
                nc.vector.tensor_tensor(out=res[:, :], in0=hp[rows, 72:80],
                                        in1=hiF[:, :], op=Alu.subtract)
                nc.vector.tensor_copy(
                    out=sdw1[:, wi * 2 * H1 + H1:(wi + 1) * 2 * H1],
                    in_=res[:, :])
        nc.sync.dma_start(
            out=bass.AP(tensor=t1h_d[:, :].tensor,
                        offset=tc_ * NCH * P * RLB,
                        ap=[[RLB, P], [P * RLB, NCH], [1, RLB]]),
            in_=st[:, :])

    if abl != "nocoll":
        nc.gpsimd.collective_compute(
            "AllGather", Alu.bypass, replica_groups=groups,
            ins=[t1h_d[:, :]], outs=[tab1_d[:, :]])

    # ---------------- layer-1 drain chunk (softmax div + bias + ELU) ----
    def flush1(chunk, ck, c):
        c0 = c * DCH * W
        den = dp.tile([H1, DCH * W], f32, tag="den")
        nc.vector.tensor_scalar_add(den[:, :ck], chunk[64:72, :ck], 1e-10)
        inv = dp.tile([H1, DCH * W], f32, tag="inv")
        nc.vector.reciprocal(inv[:, :ck], den[:, :ck])
        pb = np_.tile([W, DCH * W], f32, tag="pb")
        nc.tensor.matmul(out=pb[:, :ck], lhsT=E8[:, :], rhs=inv[:, :ck],
                         start=True, stop=True)
        ot = dp.tile([W, DCH * W], f32, tag="ot")
        nc.vector.tensor_tensor(out=ot[:, :ck], in0=chunk[:64, :ck],
                                in1=pb[:, :ck], op=Alu.mult)
        nc.vector.tensor_tensor(out=ot[:, :ck], in0=ot[:, :ck],
                                in1=b1c[:, :1].to_broadcast([W, ck]),
                                op=Alu.add)
        ex = dp.tile([W, DCH * W], f32, tag="ex")
        nc.scalar.activation(out=ex[:, :ck], in_=ot[:, :ck], func=Act.Exp)
        nc.vector.tensor_scalar(out=ex[:, :ck], in0=ex[:, :ck], scalar1=-1.0,
                                scalar2=0.0, op0=Alu.add, op1=Alu.min)
        rl = dp.tile([W, DCH * W], f32, tag="rl")
        nc.vector.tensor_scalar_max(rl[:, :ck], ot[:, :ck], 0.0)
        nc.vector.tensor_tensor(out=h2T[:, c0:c0 + ck], in0=ex[:, :ck],
                                in1=rl[:, :ck], op=Alu.add)

    # ---------------- layer-2 drain chunk (softmax div + bias + store) ----
    def flush2(chunk, ck, c):
        ns = ck // 65
        den2 = dp.tile([W, DCH], f32, tag="den2")
        nc.vector.tensor_scalar_add(
            den2[:, :ns], _ap(chunk[:, :], 64, [[65, ns]]), 1e-10)
        inv2 = dp.tile([W, DCH], f32, tag="inv2")
        nc.vector.reciprocal(inv2[:, :ns], den2[:, :ns])
        ob = dp.tile([W, DCH * OUT_DIM], f32, tag="ob")
        nc.vector.tensor_tensor(
            out=_ap(ob[:, :], 0, [[OUT_DIM, ns], [1, OUT_DIM]]),
            in0=_ap(chunk[:, :], 0, [[65, ns], [1, OUT_DIM]]),
            in1=_ap(inv2[:, :], 0, [[1, ns], [0, OUT_DIM]]),
            op=Alu.mult)
        nc.vector.tensor_tensor(
            out=_ap(ob[:, :], 0, [[OUT_DIM, ns], [1, OUT_DIM]]),
            in0=_ap(ob[:, :], 0, [[OUT_DIM, ns], [1, OUT_DIM]]),
            in1=_ap(b2r[:, :], 0, [[0, ns], [1, OUT_DIM]]),
            op=Alu.add)
        nc.sync.dma_start(
            out=bass.AP(tensor=out_d[:, :].tensor,
                        offset=c * DCH * W * OUT_DIM,
                        ap=[[OUT_DIM, W], [W * OUT_DIM, ns], [1, OUT_DIM]]),
            in_=ob[:, :ns * OUT_DIM])

    # ---------------- edge phase ----------------
    def edge_phase(layer, tab, H, GWm, sdw, scol, flush):
        chunk = None
        for si, (sl0, nsl, bb0, nlo, nhi) in enumerate(supers):
            nblk = nlo + nhi
            js = super_js[si]
            G = gp.tile([P, nblk * RLB], bf16, tag="G")

            def out3(apx, k):
                return bass.AP(tensor=apx.tensor, offset=apx.offset,
                               ap=[list(apx.ap[0])] + [[RLB, k], [1, RLB]])

            def gathers(col0, nb, tab_slice):
                for off in range(0, nb, GCH):
                    k = min(GCH, nb - off)
                    a = col0 + off
                    nc.gpsimd.dma_gather(
                        out_ap=out3(G[:, a * RLB:(a + k) * RLB], k),
                        in_ap=tab_slice,
                        idxs_ap=idx[:, (bb0 + a) * 8:(bb0 + a + k) * 8],
                        num_idxs=k * P, num_idxs_reg=k * P, elem_size=RLB)

            if abl != "nogather":
                if nlo:
                    gathers(0, nlo, tab[0:HALF, :])
                if nhi:
                    gathers(nlo, nhi, tab[HALF:, :])

            # host-precomputed one-hots, streamed from DRAM
            ohT = otp.tile([W, nblk * P], bf16, tag="ohT")
            nc.sync.dma_start(out=ohT[:, :],
                              in_=ohT_d[:, bb0 * P:(bb0 + nblk) * P])
            oh = op_.tile([P, nblk * W], bf16, tag="oh")
            nc.sync.dma_start(out=oh[:, :],
                              in_=oh_d[:, bb0 * W:(bb0 + nblk) * W])

            # per-edge sD via transposed-one-hot matmuls ([hi | lo] rhs)
            psD = sp.tile([P, nblk * 2 * H], f32, tag="psD")
            if abl != "nopsd":
                for j in range(nblk):
                    s = block_slot[bb0 + j]
                    nc.tensor.matmul(
                        out=psD[:, j * 2 * H:(j + 1) * 2 * H],
                        lhsT=ohT[:, j * P:(j + 1) * P],
                        rhs=sdw[:, s * 2 * H:(s + 1) * 2 * H],
                        start=True, stop=True)

            # e = sS + sD_hi + sD_lo ; lrelu ; p = exp -> G score cols (bf16)
            Gf = G[:, :].bitcast(f32)
            sS = bass.AP(tensor=Gf.tensor, offset=Gf.offset + 32,
                         ap=[list(Gf.ap[0])] + [[RLB // 2, nblk], [1, H]])
            eS = wp.tile([P, nblk * H], f32, tag="eS")
            if abl == "nopsd":
                nc.vector.tensor_copy(out=eS[:, :], in_=sS)
            else:
                nc.vector.tensor_tensor(
                    out=eS[:, :], in0=sS,
                    in1=_ap(psD[:, :], 0, [[2 * H, nblk], [1, H]]), op=Alu.add)
                nc.vector.tensor_tensor(
                    out=eS[:, :], in0=eS[:, :],
                    in1=_ap(psD[:, :], H, [[2 * H, nblk], [1, H]]), op=Alu.add)
            nc.vector.scalar_tensor_tensor(
                out=eS[:, :], in0=eS[:, :], scalar=NEG_SLOPE,
                in1=eS[:, :], op0=Alu.mult, op1=Alu.max)
            p_dst = _ap(G[:, :], W, [[RLB, nblk], [1, H]])
            nc.scalar.activation(out=p_dst, in_=eS[:, :], func=Act.Exp)

            # msg = h * p (per-head broadcast)
            if H == 1:
                m_ap = _ap(G[:, :], 0, [[RLB, nblk], [1, W]])
                p_ap = _ap(G[:, :], W, [[RLB, nblk], [0, W]])
            else:
                m_ap = _ap(G[:, :], 0, [[RLB, nblk], [H, H], [1, 64 // H]])
                p_ap = _ap(G[:, :], W, [[RLB, nblk], [1, H], [0, 64 // H]])
            nc.vector.tensor_tensor(out=m_ap, in0=m_ap, in1=p_ap, op=Alu.mult)

            # scatter per slot; stage into drain chunks of DCH slots
            for s in range(sl0, sl0 + nsl):
                jl = js[s]
                if layer == 1:
                    ps = pp.tile([64 + H, W], f32, tag="ps")
                    for k, j in enumerate(jl):
                        nc.tensor.matmul(
                            out=ps[:, :],
                            lhsT=_ap(G[:, :], j * RLB, [[1, GWm]]),
                            rhs=oh[:, j * W:(j + 1) * W],
                            start=(k == 0), stop=(k == len(jl) - 1))
                    rows = 64 + H
                else:
                    ps = pp.tile([W, GWm], f32, tag="ps")
                    for k, j in enumerate(jl):
                        nc.tensor.matmul(
                            out=ps[:, :],
                            lhsT=oh[:, j * W:(j + 1) * W],
                            rhs=_ap(G[:, :], j * RLB, [[1, GWm]]),
                            start=(k == 0), stop=(k == len(jl) - 1))
                    rows = W
                if chunk is None:
                    chunk = dp.tile([rows, DCH * scol], f32,
                                    tag=f"stage{layer}")
                sloc = s % DCH
                nc.vector.tensor_copy(
                    out=chunk[:, sloc * scol:(sloc + 1) * scol], in_=ps[:, :])
                if sloc == DCH - 1 or s == wpc - 1:
                    flush(chunk, (sloc + 1) * scol, s // DCH)
                    chunk = None

    edge_phase(1, tab1_d, H1, 64 + H1, sdw1, W, flush1)

    # ---------------- layer-2 node phase ----------------
    for tc_ in range(nt // NCH):
        st2 = wp.tile([P, NCH * RLB], bf16, tag="st2")
        for q in range(NCH):
            t = tc_ * NCH + q
            hp2 = np_.tile([P, 80], f32, tag="hp")
            nc.tensor.matmul(out=hp2[:, :66], lhsT=h2T[:, t * P:(t + 1) * P],
                             rhs=wc2[:, :], start=True, stop=True)
            nc.vector.tensor_copy(out=st2[:, q * RLB:q * RLB + W],
                                  in_=hp2[:, :W])
            stf2 = st2[:, :].bitcast(f32)
            nc.vector.tensor_copy(
                out=bass.AP(tensor=stf2.tensor,
                            offset=stf2.offset + q * (RLB // 2) + 32,
                            ap=[list(stf2.ap[0])] + [[1, 1]]),
                in_=hp2[:, W:W + 1])
            nc.vector.memset(st2[:, q * RLB + W + 2:(q + 1) * RLB], 0.0)
            for wi, rows in ((2 * t, slice(0, W)), (2 * t + 1, slice(W, P))):
                hi = sdw2[:, wi * 2:wi * 2 + 1]
                nc.vector.tensor_copy(out=hi, in_=hp2[rows, 65:66])
                hiF = wp.tile([W, 1], f32, tag="hiF2")
                nc.vector.tensor_copy(out=hiF[:, :], in_=hi)
                res = wp.tile([W, 1], f32, tag="res2")
                nc.vector.tensor_tensor(out=res[:, :], in0=hp2[rows, 65:66],
                                        in1=hiF[:, :], op=Alu.subtract)
                nc.vector.tensor_copy(out=sdw2[:, wi * 2 + 1:wi * 2 + 2],
                                      in_=res[:, :])
        nc.sync.dma_start(
            out=bass.AP(tensor=t2h_d[:, :].tensor,
                        offset=tc_ * NCH * P * RLB,
                        ap=[[RLB, P], [P * RLB, NCH], [1, RLB]]),
            in_=st2[:, :])

    if abl != "nocoll":
        nc.gpsimd.collective_compute(
            "AllGather", Alu.bypass, replica_groups=groups,
            ins=[t2h_d[:, :]], outs=[tab2_d[:, :]])

    edge_phase(2, tab2_d, 1, 65, sdw2, 65, flush2)


def kernel(**inputs):
    import concourse.bacc as bacc
    from concourse.bass_utils import run_bass_kernel_spmd

    n = inputs["x"].shape[0]
    cfg, in_maps, perm = _prep(**inputs)

    nc = bacc.Bacc("TRN2", target_bir_lowering=False, debug=False,
                   num_devices=cfg["n_cores"])
    _build(nc, cfg)
    nc.compile()

    res = run_bass_kernel_spmd(nc, in_maps,
                               core_ids=list(range(cfg["n_cores"])))
    full = np.concatenate([r["out"] for r in res.results], axis=0)
    out = full[perm[:n]]
    return np.ascontiguousarray(out, np.float32)
